# revision 82
# baseline (speedup 1.0000x reference)
"""Trainium2 Bass kernel for IntraFrameNet (self-attention + conv head).

Math (per sample b):
  f = curr_features[b].reshape(C, M)                      # C=128, M=4096
  S = f^T f * C^-0.5   (symmetric, [M, M])
  P = softmax(S, axis=-1)
  feats1 = f @ P^T   ([C, M]);  x = [feats1; f]           # [2C, M]
  y = W1 @ x + b1 -> BN(inference) -> leaky_relu(0.01)
  pred = w2 @ y + b2                                      # [1, M]

Design (1 sample / core, 8 cores, data-parallel; bf16 compute):
  - S is SYMMETRIC, so exp(S) is too.  Processing is column-super-major
    (4 supers of 1024); only tiles with row-super >= col-super (10/16)
    are S-matmul'ed + exp-ed (ScalarE is the limiting engine at ~1
    elem/cycle/lane).  The mirrored 6/16 are produced by xbar DMA
    transposes (dma_start_transpose, 16x128 tiles) of the direct exp
    tiles into per-block "mirror banks" -- this costs idle DMA-engine
    time instead of PE/DVE time.  out[p, j, c] = in[c, j*128+p] gives
    each [128,1024] mirror chunk as a strided slice, consumed directly
    as a matmul moving operand.
  - exp uses a constant input shift (softmax-invariant) for bf16 range
    comfort.  fp8 P / DoubleRow was evaluated and is impossible here:
    the reference data has near-duplicate feature columns, so off-diag
    logits span exp(21+), and any per-row rescale breaks the transpose
    symmetry this design depends on.
  - Softmax denominators are assembled in pD[128, chunk, 4] (one slot
    per column-super) entirely by dummy-dest DVE tensor_scalar accums
    (2x mode; tensor_reduce runs 1x, and the ACT accum_out aux costs
    187ns on the critical ScalarE path).
  - PV accumulates per column in one PSUM ot tile over all 32 chunks
    (direct pair tiles + mirror-bank slices); on completion the tile is
    snapshotted to SBUF by an ACT copy so the single PSUM ot slot frees
    for the next column without waiting on the head's Dinv chain.
    Softmax division is deferred: feats1 = ot * Dinv_bcast (Dinv
    broadcast by a selector matmul), then y = W1a @ feats1 + W1b @ f
    with BN folded on host.
  - The head runs as 3 stages software-pipelined into the next column;
    PE fill work (backlog'd PVs, mirror D-sums) is emitted BEFORE each
    pair's S-matmuls because engines execute strictly in order -- a
    WAR-blocked S must never sit ahead of ready work in the PE queue.
  - PSUM: 4 banks S->exp double buffer, 2 banks ot, 2 banks head tiles.
"""

import numpy as np
import ml_dtypes

import concourse.bass as bass
from concourse import bacc
import concourse.mybir as mybir
import concourse.tile as tile
from concourse.bass_utils import run_bass_kernel_spmd
from concourse.masks import make_identity

B, C, H, W = 8, 128, 64, 64
M = H * W            # 4096
NCH = 32             # row chunks of 128
SUP = 1024           # cols per super-block
NSUP = 4
CPS = 8              # chunks per super
PPS = 4              # chunk-pairs per super
NPAIR = 16
SCALE = float(C) ** -0.5
EXP_SHIFT = -4.0     # exp(z + shift): softmax-invariant, tames bf16 magnitudes
BN_EPS = 1e-5
LEAKY = 0.01

PV_F8 = False        # fp8 PV is impossible for this data (logit span e^21)
S_F8 = False         # fp8e4 DoubleRow for the S matmuls (quantizes f only)

f32 = mybir.dt.float32
bf16 = mybir.dt.bfloat16
f8 = mybir.dt.float8e5
PDT = f8 if PV_F8 else bf16
NP_PDT = ml_dtypes.float8_e5m2 if PV_F8 else ml_dtypes.bfloat16
AF = mybir.ActivationFunctionType
ALU = mybir.AluOpType
DRMODE = mybir.MatmulPerfMode.DoubleRow


def _build():
    nc = bacc.Bacc("TRN2", target_bir_lowering=False)

    fb_d = nc.dram_tensor("fb", [C, M], bf16, kind="ExternalInput")
    fTb_d = nc.dram_tensor("fTb", [128, NCH * 128], bf16, kind="ExternalInput")
    if PV_F8:
        fT8_d = nc.dram_tensor("fT8", [128, NCH * 128], f8, kind="ExternalInput")
    if S_F8:
        f8c_d = nc.dram_tensor("f8c", [64, 2 * M], mybir.dt.float8e4,
                               kind="ExternalInput")
    w1aT_d = nc.dram_tensor("w1aT", [C, C], f32, kind="ExternalInput")
    w1bT_d = nc.dram_tensor("w1bT", [C, C], f32, kind="ExternalInput")
    bhead_d = nc.dram_tensor("bhead", [C, 1], f32, kind="ExternalInput")
    w2T_d = nc.dram_tensor("w2T", [C, 1], f32, kind="ExternalInput")
    sel_d = nc.dram_tensor("sel", [CPS, CPS * 128], f32, kind="ExternalInput")
    pred_d = nc.dram_tensor("pred", [1, M], f32, kind="ExternalOutput")

    with tile.TileContext(nc) as tc:
        with (
            tc.tile_pool(name="singles", bufs=1) as singles,
            tc.tile_pool(name="pbp", bufs=7) as pbp,
            tc.tile_pool(name="pbmp", bufs=1) as pbmp,
            tc.tile_pool(name="sbm", bufs=2) as sbm,
            tc.tile_pool(name="ps_st", bufs=2, space="PSUM") as ps_st,
            tc.tile_pool(name="ps_ot", bufs=1, space="PSUM") as ps_ot,
            tc.tile_pool(name="ps_tr", bufs=2, space="PSUM") as ps_tr,
        ):
            # ---------------- loads ----------------
            fb = singles.tile([C, M], bf16)
            for k, q in enumerate((0, 1)):
                eng = nc.sync if k % 2 == 0 else nc.scalar
                eng.dma_start(
                    out=fb[:, q * 512 : (q + 1) * 512],
                    in_=fb_d[:, q * 512 : (q + 1) * 512],
                )
            fTb = singles.tile([128, NCH, 128], bf16)
            nc.gpsimd.dma_start(out=fTb[:, 0:8, :], in_=fTb_d[:, 0:1024])
            if S_F8:
                f8c = singles.tile([64, 2, M], mybir.dt.float8e4)
                nc.scalar.dma_start(out=f8c[:, :, :], in_=f8c_d[:, :])
            if PV_F8:
                fT8 = singles.tile([128, NCH, 128], f8)
            for k, q in enumerate((2, 3, 4, 5, 6, 7)):
                eng = nc.sync if k % 2 == 0 else nc.scalar
                eng.dma_start(
                    out=fb[:, q * 512 : (q + 1) * 512],
                    in_=fb_d[:, q * 512 : (q + 1) * 512],
                )
            for p in (1, 2, 3):
                nc.gpsimd.dma_start(
                    out=fTb[:, p * 8 : (p + 1) * 8, :],
                    in_=fTb_d[:, p * 1024 : (p + 1) * 1024],
                )
            if PV_F8:
                for p in range(4):
                    nc.gpsimd.dma_start(
                        out=fT8[:, p * 8 : (p + 1) * 8, :],
                        in_=fT8_d[:, p * 1024 : (p + 1) * 1024],
                    )
            w1aT_f = singles.tile([C, C], f32)
            nc.gpsimd.dma_start(out=w1aT_f, in_=w1aT_d[:, :])
            w1bT_f = singles.tile([C, C], f32)
            nc.gpsimd.dma_start(out=w1bT_f, in_=w1bT_d[:, :])
            bhead = singles.tile([C, 1], f32)
            nc.gpsimd.dma_start(out=bhead, in_=bhead_d[:, :])
            w2T_f = singles.tile([C, 1], f32)
            nc.gpsimd.dma_start(out=w2T_f, in_=w2T_d[:, :])
            sel_f = singles.tile([CPS, CPS * 128], f32)
            nc.gpsimd.dma_start(out=sel_f, in_=sel_d[:, :])

            # identities (gpsimd) + small converts (DVE)
            ident_f32 = singles.tile([128, 128], f32)
            make_identity(nc, ident_f32)
            w1aT_b = singles.tile([C, C], bf16)
            nc.vector.tensor_copy(out=w1aT_b, in_=w1aT_f)
            w1bT_b = singles.tile([C, C], bf16)
            nc.vector.tensor_copy(out=w1bT_b, in_=w1bT_f)
            w2T_b = singles.tile([C, 1], bf16)
            nc.vector.tensor_copy(out=w2T_b, in_=w2T_f)
            sel_b = singles.tile([CPS, CPS * 128], bf16)
            nc.vector.tensor_copy(out=sel_b, in_=sel_f)

            pD = singles.tile([128, NCH, NSUP], f32)
            pred_sb = singles.tile([1, M], f32)
            shift_ap = singles.tile([128, 1], f32)
            nc.gpsimd.memset(shift_ap, EXP_SHIFT)
            scratch = singles.tile([128, CPS, 128], bf16)

            # ---------------- helpers ----------------
            pb_tiles = {}     # u -> current-column direct pair tile
            mbanks = {}       # (src_col, dest_col) -> xbar-transposed block
            pending = []      # deferred mirror D-reduce jobs
            dmaq = [nc.sync, nc.scalar]
            backlog = []      # deferred PV contributions (u, rhs, col)
            ot_tiles = {}     # col -> PSUM accumulation tile
            pvn = {}          # col -> emitted PV count
            pv_target = {0: NPAIR, 1: NPAIR, 2: NPAIR - PPS, 3: NPAIR - PPS}
            pre_acc = {}      # dest col -> psum accumulators (col0 prefold)
            pre_sb = {}       # dest col -> SBUF snapshots
            otsb_tiles = {}   # col -> SBUF snapshot of completed ot
            head_state = {}   # g -> inter-stage tiles

            def emit_s_exp(u, b, q):
                """S matmuls + exp for chunk t = 2u+q of column b."""
                t = 2 * u + q
                st = ps_st.tile([128, SUP], f32, tag="st", name=f"st{b}_{t}")
                for h in range(2):
                    if S_F8:
                        nc.tensor.matmul(
                            st[:, h * 512 : (h + 1) * 512],
                            lhsT=f8c[:, :, t * 128 : (t + 1) * 128],
                            rhs=f8c[:, :, b * SUP + h * 512 : b * SUP + (h + 1) * 512],
                            start=True,
                            stop=True,
                            perf_mode=DRMODE,
                        )
                    else:
                        nc.tensor.matmul(
                            st[:, h * 512 : (h + 1) * 512],
                            lhsT=fb[:, t * 128 : (t + 1) * 128],
                            rhs=fb[:, b * SUP + h * 512 : b * SUP + (h + 1) * 512],
                            start=True,
                            stop=True,
                        )
                if q == 0:
                    diag = u // PPS == b
                    dt = bf16 if (diag or not PV_F8) else f8
                    pb_tiles[u] = pbp.tile(
                        [128, 2, SUP], dt,
                        tag="pbd" if diag else "pb",
                        bufs=3 if diag else None,
                        name=f"pb{b}_{u}",
                    )
                nc.scalar.activation(
                    out=pb_tiles[u][:, q, :],
                    in_=st,
                    func=AF.Exp,
                    scale=SCALE,
                    bias=shift_ap,
                )
                if True:
                    nc.vector.tensor_scalar(
                        out=scratch,
                        in0=pb_tiles[u][:, q, :],
                        scalar1=0.0,
                        scalar2=0.0,
                        op0=ALU.add,
                        op1=ALU.add,
                        accum_out=pD[:, t, b : b + 1],
                    )
                a_sup = u // PPS
                if a_sup > b and q == 1:
                    # mirror path: one xbar DMA transposes the whole pair into
                    # the (b -> a_sup) bank: mb[p, j2, jl, c] = src_j2[c, jl*128+p]
                    if (b, a_sup) not in mbanks:
                        mbanks[(b, a_sup)] = pbmp.tile(
                            [128, CPS, CPS, 128], bf16,
                            tag=f"m{b}_{a_sup}", name=f"mb{b}_{a_sup}",
                        )
                    j2 = 2 * u - a_sup * CPS
                    dmaq[0].dma_start_transpose(
                        mbanks[(b, a_sup)][:, j2 : j2 + 2, :, :],
                        pb_tiles[u][:, :, :],
                    )

            def emit_pv(u, rhs_pair, b):
                """PV contribution of chunk pair u into ot[b].

                rhs_pair: direct pair tile [128, 2, SUP], or an mbank for
                mirrors (4D [128, CPS, CPS, 128])."""
                ot = ot_tiles[b]
                n = pvn[b]
                last = n == pv_target[b] - 1
                mirror = len(rhs_pair.shape) == 4
                for h in range(2):
                    for q in range(2):
                        t = 2 * u + q
                        if mirror:
                            rhs = rhs_pair[:, h * 4 : (h + 1) * 4, t % CPS, :]
                        else:
                            rhs = rhs_pair[:, q, h * 512 : (h + 1) * 512]
                        nc.tensor.matmul(
                            ot[:, h * 512 : (h + 1) * 512],
                            lhsT=fTb[:, t, :],
                            rhs=rhs,
                            start=(n == 0 and q == 0),
                            stop=(last and q == 1),
                        )
                pvn[b] += 1
                if pvn[b] == pv_target[b]:
                    # snapshot ot to SBUF: frees the PSUM tile for the next
                    # column without waiting for the head's Dinv chain.  For
                    # prefolded columns the snapshot IS the fold-in add.
                    otp = ot_tiles.pop(b)
                    otsb = sbm.tile([128, SUP], f32, tag="otsb", name=f"otsb{b}")
                    if b in pre_sb:
                        for h in range(2):
                            nc.vector.tensor_tensor(
                                out=otsb[:, h * 512 : (h + 1) * 512],
                                in0=otp[:, h * 512 : (h + 1) * 512],
                                in1=pre_sb[b][h],
                                op=ALU.add,
                            )
                    else:
                        nc.scalar.activation(
                            out=otsb, in_=otp, func=AF.Copy, scale=1.0
                        )
                    otsb_tiles[b] = otsb

            def emit_mirror_job(job):
                """Row-sum of one mirror chunk into its pD slot.  A dummy-dest
                tensor_scalar runs in the DVE 2x mode (a tensor_reduce would
                run at 1x and cost ~2x as much)."""
                tau, acol, mb, _b_src, _ci = job
                nc.vector.tensor_scalar(
                    out=scratch,
                    in0=mb[:, :, tau % CPS, :],
                    scalar1=0.0,
                    scalar2=0.0,
                    op0=ALU.add,
                    op1=ALU.add,
                    accum_out=pD[:, tau, acol : acol + 1],
                )

            def emit_head1(g):
                """Head stage 1: softmax denominators (DVE only)."""
                Dg = sbm.tile([128, CPS], f32, tag="Dg", name=f"Dg{g}")
                nc.vector.tensor_reduce(
                    out=Dg,
                    in_=pD[:, CPS * g : CPS * (g + 1), :],
                    axis=mybir.AxisListType.X,
                    op=ALU.add,
                )
                Dinvg = sbm.tile([128, CPS], f32, tag="Dinv", name=f"Dinv{g}")
                nc.vector.reciprocal(out=Dinvg, in_=Dg)
                head_state[g] = [Dinvg]

            def emit_head2(g):
                emit_head2a(g)
                emit_head2b(g)

            def emit_head2a(g):
                """Head stage 2a: Dinv transpose + broadcast (no ot dep)."""
                (Dinvg,) = head_state[g]
                drpg = ps_tr.tile([CPS, 128], f32, tag="tr", name=f"drpg{g}")
                nc.tensor.transpose(drpg, Dinvg, ident_f32)
                DrowTg = sbm.tile([CPS, 128], bf16, tag="DrowT", name=f"DrowT{g}")
                nc.vector.tensor_copy(out=DrowTg, in_=drpg)
                dbps = []
                for h in range(2):
                    dbp = ps_tr.tile([128, 512], f32, tag="tr", name=f"dbp{g}_{h}")
                    for j in range(4):
                        jj = h * 4 + j
                        nc.tensor.matmul(
                            dbp[:, j * 128 : (j + 1) * 128],
                            lhsT=sel_b[:, jj * 128 : (jj + 1) * 128],
                            rhs=DrowTg,
                            start=True,
                            stop=True,
                        )
                    dbps.append(dbp)
                head_state[g] = dbps

            def emit_head2b(g):
                """Head stage 2b: feats1 norm + conv1 (reads the ot snapshot)."""
                dbps = head_state[g]
                ot = otsb_tiles.pop(g)
                fnorms = []
                for h in range(2):
                    src = ot[:, h * 512 : (h + 1) * 512]
                    fnorm = sbm.tile([128, 512], bf16, tag="fnorm", name=f"fnorm{g}_{h}")
                    nc.vector.tensor_tensor(
                        out=fnorm,
                        in0=src,
                        in1=dbps[h],
                        op=ALU.mult,
                    )
                    fnorms.append(fnorm)
                yps = []
                for h in range(2):
                    base = g * SUP + h * 512
                    yp = ps_tr.tile([128, 512], f32, tag="tr", name=f"yp{g}_{h}")
                    nc.tensor.matmul(yp, lhsT=w1aT_b, rhs=fnorms[h], start=True, stop=False)
                    nc.tensor.matmul(
                        yp,
                        lhsT=w1bT_b,
                        rhs=fb[:, base : base + 512],
                        start=False,
                        stop=True,
                    )
                    yps.append(yp)
                head_state[g] = yps

            def emit_head3(g):
                """Head stage 3: bias+leaky (ACT, same table set as exp),
                final 1-channel conv, pred copy."""
                yps = head_state.pop(g)
                zsbs = []
                for h in range(2):
                    zsb = sbm.tile([128, 512], bf16, tag="zsb", name=f"zsb{g}_{h}")
                    if g == NSUP - 1:
                        nc.scalar.activation(
                            out=zsb, in_=yps[h], func=AF.Prelu, bias=bhead,
                            scale=1.0, alpha=LEAKY,
                        )
                    else:
                        t1 = sbm.tile([128, 512], f32, tag="t1", name=f"t1_{g}_{h}")
                        nc.vector.tensor_scalar_add(out=t1, in0=yps[h], scalar1=bhead)
                        nc.vector.scalar_tensor_tensor(
                            out=zsb, in0=t1, scalar=LEAKY, in1=t1,
                            op0=ALU.mult, op1=ALU.max,
                        )
                    zsbs.append(zsb)
                pps = []
                for h in range(2):
                    pp = ps_tr.tile([1, 512], f32, tag="tr", name=f"pp{g}_{h}")
                    nc.tensor.matmul(pp, lhsT=w2T_b, rhs=zsbs[h], start=True, stop=True)
                    pps.append(pp)
                for h in range(2):
                    base = g * SUP + h * 512
                    nc.vector.tensor_copy(
                        out=pred_sb[0:1, base : base + 512], in_=pps[h]
                    )


            # ---------------- main loop ----------------
            # Per column-super b: direct subgroups processed in DESCENDING
            # super order (3, 2, .., b) so that mirror-transpose jobs are
            # created early and drain during the column (never a tail burst).
            for b in range(NSUP):
                pairs = [
                    a * PPS + k
                    for a in range(NSUP - 1, b - 1, -1)
                    for k in range(PPS)
                ]
                mir = [(v, mbanks[(v // PPS, b)])
                       for v in range((PPS if b >= 2 else 0), PPS * b)]
                nd, nm = len(pairs), len(mir)
                head_at = (1 if nd <= 4 else 2) if b > 0 else 0
                mi = 0
                for i, u in enumerate(pairs):
                    a_sup = u // PPS
                    # ---- fill work first (runs while S waits on st WAR) ----
                    for _ in range(1 if len(pending) < 6 else 2):
                        if pending and pending[0][4] < i - 1:
                            emit_mirror_job(pending.pop(0))
                    while backlog and backlog[0][2] < b:
                        du, rhs, dcol = backlog.pop(0)
                        emit_pv(du, rhs, dcol)
                    if b > 0 and i == (0 if head_at == 1 else 1):
                        emit_head1(b - 1)
                    if b > 0 and i == head_at:
                        emit_head2(b - 1)
                    if b > 0 and i == min(head_at + 2, nd - 1):
                        emit_head3(b - 1)
                    if i == 0:
                        pvn[b] = 0
                        ot_tiles[b] = ps_ot.tile(
                            [C, SUP], f32, tag="ot", name=f"ot{b}"
                        )
                    if b in ot_tiles:
                        cap = max(2, -(-nm // nd))
                        want = min(((i + 1) * nm) // nd, mi + cap)
                        while mi < want:
                            v, pbm = mir[mi]
                            emit_pv(v, pbm, b)
                            mi += 1
                        while len(backlog) > 2:
                            du, rhs, dcol = backlog.pop(0)
                            emit_pv(du, rhs, dcol)
                    if b == 0 and 8 <= i <= 15:
                        # cols 2/3's super-0 mirror PVs, prefolded into spare
                        # head-pool PSUM while PE is otherwise ACT-bound
                        dst = 3 if i <= 11 else 2
                        v = (i - 8) % PPS
                        mb0d = mbanks[(0, dst)]
                        if v == 0:
                            pre_acc[dst] = [
                                ps_tr.tile([128, 512], f32, tag="tr",
                                           name=f"pre{dst}_{h}")
                                for h in range(2)
                            ]
                        for h in range(2):
                            for q in range(2):
                                t3 = 2 * v + q
                                nc.tensor.matmul(
                                    pre_acc[dst][h],
                                    lhsT=fTb[:, t3, :],
                                    rhs=mb0d[:, h * 4 : (h + 1) * 4, t3 % CPS, :],
                                    start=(v == 0 and q == 0),
                                    stop=(v == PPS - 1 and q == 1),
                                )
                        if v == PPS - 1:
                            pre_sb[dst] = []
                            for h in range(2):
                                sb3 = sbm.tile(
                                    [128, 512], f32, tag=f"presb{dst}",
                                    name=f"pre_sb{dst}_{h}",
                                )
                                nc.vector.tensor_copy(
                                    out=sb3, in_=pre_acc[dst][h]
                                )
                                pre_sb[dst].append(sb3)
                    # ---- this pair's S + exp (the ACT critical path) ----
                    for q in range(2):
                        emit_s_exp(u, b, q)
                    backlog.append((u, pb_tiles[u], b))
                    # subgroup complete -> queue mirror D-reduce jobs
                    if u % PPS == PPS - 1 and a_sup > b:
                        mb = mbanks[(b, a_sup)]
                        for tau in range(CPS * b, CPS * b + CPS):
                            pending.append((tau, a_sup, mb, b, i))
                # leftover mirror PVs carry over like direct ones
                while pending:
                    emit_mirror_job(pending.pop(0))
                while mi < nm:
                    v, pbm = mir[mi]
                    backlog.append((v, pbm, b))
                    mi += 1
            emit_head1(NSUP - 1)
            emit_head2a(NSUP - 1)
            while backlog:
                du, rhs, dcol = backlog.pop(0)
                emit_pv(du, rhs, dcol)
            emit_head2b(NSUP - 1)
            emit_head3(NSUP - 1)
            nc.sync.dma_start(out=pred_d[:, :], in_=pred_sb)

    nc.finalize()
    return nc


_NC = None


def _get_nc():
    global _NC
    if _NC is None:
        _NC = _build()
    return _NC


def _prep_host(inputs):
    curr = np.asarray(inputs["curr_features"], np.float32)
    w1 = np.asarray(inputs["w1"], np.float32)
    b1 = np.asarray(inputs["b1"], np.float32)
    gamma = np.asarray(inputs["gamma"], np.float32)
    beta = np.asarray(inputs["beta"], np.float32)
    rm = np.asarray(inputs["running_mean"], np.float32)
    rv = np.asarray(inputs["running_var"], np.float32)
    w2 = np.asarray(inputs["w2"], np.float32)

    # fold BN (inference) into the first conv
    a = gamma / np.sqrt(rv + BN_EPS)
    W1f = w1 * a[:, None]
    bhead = (b1 * a + beta - rm * a).astype(np.float32).reshape(C, 1)
    w1aT = np.ascontiguousarray(W1f[:, :C].T, np.float32)
    w1bT = np.ascontiguousarray(W1f[:, C:].T, np.float32)
    w2T = np.ascontiguousarray(w2.T, np.float32)

    selm = np.zeros((CPS, CPS * 128), np.float32)
    for k in range(CPS):
        selm[k, k * 128 : (k + 1) * 128] = 1.0

    in_maps = []
    for s in range(B):
        f = np.ascontiguousarray(curr[s].reshape(C, M))
        fbh = f.astype(ml_dtypes.bfloat16)
        fTh = np.ascontiguousarray(f.T.reshape(NCH, 128, C).transpose(1, 0, 2))
        m = {}
        if S_F8:
            m["f8c"] = np.ascontiguousarray(
                f.reshape(2, 64, M).transpose(1, 0, 2)
                .astype(ml_dtypes.float8_e4m3fn).reshape(64, 2 * M)
            )
        m.update({
            "fTb": np.ascontiguousarray(
                fTh.astype(ml_dtypes.bfloat16).reshape(128, NCH * 128)
            ),
        })
        if PV_F8:
            m["fT8"] = np.ascontiguousarray(
                fTh.astype(ml_dtypes.float8_e5m2).reshape(128, NCH * 128)
            )
        in_maps.append(
            {
                **m,
                "fb": fbh,
                "w1aT": w1aT,
                "w1bT": w1bT,
                "bhead": bhead,
                "w2T": w2T,
                "sel": selm,
            }
        )
    return in_maps


def kernel(**inputs):
    b2 = np.asarray(inputs["b2"], np.float32)
    in_maps = _prep_host(inputs)
    nc = _get_nc()
    res = run_bass_kernel_spmd(nc, in_maps, core_ids=list(range(B)))
    preds = np.stack([r["pred"].reshape(1, H, W) for r in res.results], axis=0)
    return (preds + b2[0]).astype(np.float32)


if __name__ == "__main__":
    _build()
    print("build OK")
